# revision 1
# baseline (speedup 1.0000x reference)
"""GQA attention (dense_transformer) TRN2 Bass kernel — 8 NeuronCores.

Problem: b=2, s=2048, d=2048, nh=16, nkv=4, hd=128, causal GQA attention
block with RMS-normed+RoPE'd q/k and per-head q gains.

Sharding: batch DP=2 x head TP=4  ->  8 cores. Each core handles one batch
element, 4 q heads, 1 kv head. Wq/Wk/Wv column-sharded, Wo row-sharded;
partial outputs summed on host.

Per-core dataflow (all matmuls bf16 inputs, fp32 PSUM accumulate):
  1. QKV projections from host-pretransposed xT (row layout out).
  2. RMS-norm + RoPE fused into 4 DVE ops/tile via scalar_tensor_tensor
     (norm reciprocal and qg folded into the per-partition scalar).
  3. PE-transpose Q,K into [hd, s] layout.
  4. Attention per (head, 512-wide q-chunk, 128-wide k-tile):
     S^T = K-tile @ Q-chunk on PE; exp on ScalarE (1/sqrt(hd) folded in);
     causal zeroing of diagonal tiles via gpsimd affine_select;
     A@V and replicated row-sum as accumulating PE matmuls; one DVE
     reciprocal + one DVE multiply normalize (softmax denominator).
  5. Output projection -> outT partial [d, s] fp32; host transposes and
     sums TP partials.
"""

import math
import sys

if "/opt/trn_rl_repo" not in sys.path:
    sys.path.insert(0, "/opt/trn_rl_repo")

import numpy as np
import ml_dtypes

import concourse.mybir as mybir
import concourse.tile as tile
from concourse import bacc
from concourse.bass_utils import run_bass_kernel_spmd

F32 = mybir.dt.float32
BF16 = mybir.dt.bfloat16
AF = mybir.ActivationFunctionType
ALU = mybir.AluOpType

BF16NP = ml_dtypes.bfloat16
RMS_EPS = float(np.finfo(np.float32).eps)

S, D, NQ, HD = 2048, 2048, 4, 128
DQ = NQ * HD            # 512: per-core q width
NTP = 4                 # tensor-parallel ways (heads)
NB = 2                  # batch (data-parallel ways)
NCORES = 8

_NC_CACHE = {}


def build_kernel(S=S, D=D, NQ=NQ, HD=HD, num_devices=NCORES):
    DQ = NQ * HD
    NST = S // 128          # s-tiles
    NDC = D // 128          # d-chunks (projection contraction)
    NQC = S // 512          # q-chunks for attention
    scale = 1.0 / math.sqrt(HD)
    H = HD // 2

    nc = bacc.Bacc("TRN2", target_bir_lowering=False, debug=False,
                   num_devices=num_devices)

    xT = nc.dram_tensor("xT", [D, S], BF16, kind="ExternalInput").ap()
    wq = nc.dram_tensor("wq", [D, DQ], BF16, kind="ExternalInput").ap()
    wkv = nc.dram_tensor("wkv", [D, 2 * HD], BF16, kind="ExternalInput").ap()
    wo = nc.dram_tensor("wo", [DQ, D], BF16, kind="ExternalInput").ap()
    cc = nc.dram_tensor("cc", [S, HD], F32, kind="ExternalInput").ap()
    ss = nc.dram_tensor("ss", [S, HD], F32, kind="ExternalInput").ap()
    qgb = nc.dram_tensor("qgb", [128, NQ], F32, kind="ExternalInput").ap()
    ident = nc.dram_tensor("ident", [128, 128], BF16, kind="ExternalInput").ap()
    ones = nc.dram_tensor("ones", [128, 128], BF16, kind="ExternalInput").ap()
    outT = nc.dram_tensor("outT", [D, S], F32, kind="ExternalOutput").ap()

    with tile.TileContext(nc) as tc:
        from contextlib import ExitStack
        with ExitStack() as ctx:
            consts = ctx.enter_context(tc.tile_pool(name="consts", bufs=1))
            xpool = ctx.enter_context(tc.tile_pool(name="xT", bufs=1))
            wpool = ctx.enter_context(tc.tile_pool(name="w", bufs=1))
            qt_pool = ctx.enter_context(tc.tile_pool(name="qt", bufs=1))
            yt_pool = ctx.enter_context(tc.tile_pool(name="yt", bufs=1))
            v_pool = ctx.enter_context(tc.tile_pool(name="vrow", bufs=1))

            cc_sb = consts.tile([128, NST, HD], F32, tag="cc")
            ss_sb = consts.tile([128, NST, HD], F32, tag="ss")
            nc.sync.dma_start(cc_sb[:], cc.rearrange("(n p) m -> p n m", p=128))
            nc.sync.dma_start(ss_sb[:], ss.rearrange("(n p) m -> p n m", p=128))
            qgb_sb = consts.tile([128, NQ], F32, tag="qgb")
            nc.sync.dma_start(qgb_sb[:], qgb)
            ident_sb = consts.tile([128, 128], BF16, tag="ident")
            nc.sync.dma_start(ident_sb[:], ident)
            ones_sb = consts.tile([128, 128], BF16, tag="ones")
            nc.sync.dma_start(ones_sb[:], ones)
            eps_col = consts.tile([128, 1], F32, tag="eps")
            nc.gpsimd.memset(eps_col[:], RMS_EPS)

            wq_sb = wpool.tile([128, NDC, DQ], BF16, tag="wq")
            nc.sync.dma_start(wq_sb[:], wq.rearrange("(n p) m -> p n m", p=128))
            wkv_sb = wpool.tile([128, NDC, 2 * HD], BF16, tag="wkv")
            nc.sync.dma_start(wkv_sb[:], wkv.rearrange("(n p) m -> p n m", p=128))
            wo_sb = wpool.tile([128, NQ, D], BF16, tag="wo")
            nc.sync.dma_start(wo_sb[:], wo.rearrange("(n p) m -> p n m", p=128))

            xT_sb = xpool.tile([128, NDC, S], BF16, tag="xT")
            xTr = xT.rearrange("(n p) m -> p n m", p=128)
            for dc in range(NDC):
                nc.sync.dma_start(xT_sb[:, dc, :], xTr[:, dc, :])

            qt_tiles = [qt_pool.tile([128, S], BF16, name=f"qt{h}", tag=f"qt{h}")
                        for h in range(NQ)]
            kt_tile = qt_pool.tile([128, S], BF16, name="kt", tag="kt")
            yt_tiles = [yt_pool.tile([128, S], BF16, name=f"yt{h}", tag=f"yt{h}")
                        for h in range(NQ)]
            v_tiles = [v_pool.tile([128, HD], BF16, name=f"v{st}", tag=f"v{st}")
                       for st in range(NST)]

            # ---- Phase 1: projections + rms-norm + rope + transpose ----
            with (
                tc.tile_pool(name="pqkv", bufs=2, space="PSUM") as pqkv,
                tc.tile_pool(name="ptr", bufs=4, space="PSUM") as ptr,
                tc.tile_pool(name="p1tmp", bufs=3) as p1tmp,
                tc.tile_pool(name="p1stat", bufs=3) as p1stat,
            ):
                for st in range(NST):
                    pq = pqkv.tile([128, DQ], F32, tag="pq")
                    pkv = pqkv.tile([128, 2 * HD], F32, tag="pkv")
                    for dc in range(NDC):
                        lhsT = xT_sb[:, dc, st * 128:(st + 1) * 128]
                        first, last = dc == 0, dc == NDC - 1
                        nc.tensor.matmul(pq[:], lhsT, wq_sb[:, dc, :],
                                         start=first, stop=last)
                        nc.tensor.matmul(pkv[:], lhsT, wkv_sb[:, dc, :],
                                         start=first, stop=last)

                    nc.scalar.copy(v_tiles[st][:], pkv[:, HD:2 * HD])

                    sq_scratch = p1tmp.tile([128, HD], F32, tag="sqs")
                    ssq = p1stat.tile([128, NQ + 1], F32, tag="ssq")
                    for i in range(NQ + 1):
                        src = pq[:, i * HD:(i + 1) * HD] if i < NQ else pkv[:, 0:HD]
                        nc.scalar.activation(sq_scratch[:], src, AF.Square,
                                             accum_out=ssq[:, i:i + 1])
                    rms = p1stat.tile([128, NQ + 1], F32, tag="rms")
                    nc.scalar.activation(rms[:], ssq[:], AF.Sqrt,
                                         scale=1.0 / HD, bias=eps_col[:])
                    rinv = p1stat.tile([128, NQ + 1], F32, tag="rinv")
                    nc.vector.reciprocal(rinv[:], rms[:])
                    nc.vector.tensor_mul(rinv[:, 0:NQ], rinv[:, 0:NQ], qgb_sb[:])

                    cc_t = cc_sb[:, st, :]
                    ss_t = ss_sb[:, st, :]
                    for i in range(NQ + 1):
                        q_ap = pq[:, i * HD:(i + 1) * HD] if i < NQ else pkv[:, 0:HD]
                        r_ap = rinv[:, i:i + 1]
                        tt = p1tmp.tile([128, HD], F32, tag="ropet")
                        uu = p1tmp.tile([128, HD], F32, tag="ropeu")
                        nc.vector.scalar_tensor_tensor(tt[:], q_ap, r_ap, cc_t,
                                                       op0=ALU.mult, op1=ALU.mult)
                        nc.vector.scalar_tensor_tensor(uu[:, 0:H], q_ap[:, H:HD],
                                                       r_ap, ss_t[:, 0:H],
                                                       op0=ALU.mult, op1=ALU.mult)
                        nc.vector.scalar_tensor_tensor(uu[:, H:HD], q_ap[:, 0:H],
                                                       r_ap, ss_t[:, H:HD],
                                                       op0=ALU.mult, op1=ALU.mult)
                        ro = p1tmp.tile([128, HD], BF16, tag="ro")
                        nc.vector.tensor_add(ro[:], tt[:], uu[:])
                        pt = ptr.tile([128, 128], BF16, tag="ptr")
                        nc.tensor.transpose(pt[:], ro[:], ident_sb[:])
                        dst = qt_tiles[i] if i < NQ else kt_tile
                        nc.scalar.copy(dst[:, st * 128:(st + 1) * 128], pt[:])

            # ---- Phase 2: attention ----
            with (
                tc.tile_pool(name="ps", bufs=4, space="PSUM") as ps_pool,
                tc.tile_pool(name="py", bufs=2, space="PSUM") as py_pool,
                tc.tile_pool(name="pr", bufs=2, space="PSUM") as pr_pool,
                tc.tile_pool(name="ptile", bufs=4) as pt_pool,
                tc.tile_pool(name="rcp", bufs=2) as rcp_pool,
            ):
                for qc in range(NQC):
                    n_kt = 4 * qc + 4
                    for h in range(NQ):
                        qs = qt_tiles[h][:, qc * 512:(qc + 1) * 512]
                        py = py_pool.tile([128, 512], F32, tag="py")
                        pr = pr_pool.tile([128, 512], F32, tag="pr")
                        for kt in range(n_kt):
                            pss = ps_pool.tile([128, 512], F32, tag="ps")
                            nc.tensor.matmul(
                                pss[:], kt_tile[:, kt * 128:(kt + 1) * 128],
                                qs, start=True, stop=True)
                            pt = pt_pool.tile([128, 512], BF16, tag="pt")
                            nc.scalar.activation(pt[:], pss[:], AF.Exp, scale=scale)
                            if kt >= 4 * qc:  # diagonal: zero where q < k
                                nc.gpsimd.affine_select(
                                    out=pt[:], in_=pt[:], compare_op=ALU.is_ge,
                                    fill=0.0, base=qc * 512 - kt * 128,
                                    channel_multiplier=-1, pattern=[[1, 512]])
                            first, last = kt == 0, kt == n_kt - 1
                            nc.tensor.matmul(py[:], v_tiles[kt][:], pt[:],
                                             start=first, stop=last)
                            nc.tensor.matmul(pr[:], ones_sb[:], pt[:],
                                             start=first, stop=last)
                        rcp = rcp_pool.tile([128, 512], F32, tag="rcp")
                        nc.vector.reciprocal(rcp[:], pr[:])
                        nc.vector.tensor_mul(
                            yt_tiles[h][:, qc * 512:(qc + 1) * 512], py[:], rcp[:])

            # ---- Phase 3: output projection ----
            with (
                tc.tile_pool(name="po", bufs=4, space="PSUM") as po_pool,
                tc.tile_pool(name="ob", bufs=4) as ob_pool,
            ):
                for dt in range(NDC):
                    for qc in range(NQC):
                        po = po_pool.tile([128, 512], F32, tag="po")
                        for dqc in range(NQ):
                            nc.tensor.matmul(
                                po[:], wo_sb[:, dqc, dt * 128:(dt + 1) * 128],
                                yt_tiles[dqc][:, qc * 512:(qc + 1) * 512],
                                start=(dqc == 0), stop=(dqc == NQ - 1))
                        ob = ob_pool.tile([128, 512], F32, tag="ob")
                        if (dt + qc) % 2 == 0:
                            nc.scalar.copy(ob[:], po[:])
                        else:
                            nc.vector.tensor_copy(ob[:], po[:])
                        nc.sync.dma_start(
                            outT[dt * 128:(dt + 1) * 128, qc * 512:(qc + 1) * 512],
                            ob[:])

    nc.compile()
    return nc


def get_nc():
    if "nc" not in _NC_CACHE:
        _NC_CACHE["nc"] = build_kernel()
    return _NC_CACHE["nc"]


def rope_tables(S=S, HD=HD):
    f = 1.0 / (10000.0 ** (np.arange(0, HD, 2, dtype=np.float32) / HD))
    fr = np.outer(np.arange(S, dtype=np.float32), f)
    c = np.cos(fr).astype(np.float32)
    s = np.sin(fr).astype(np.float32)
    cc = np.concatenate([c, c], axis=1)
    ss = np.concatenate([s, -s], axis=1)
    return cc, ss


def make_in_maps(x, Wq, Wk, Wv, Wo, qg):
    x = np.asarray(x, np.float32)
    Wq = np.asarray(Wq, np.float32)
    Wk = np.asarray(Wk, np.float32)
    Wv = np.asarray(Wv, np.float32)
    Wo = np.asarray(Wo, np.float32)
    qg = np.asarray(qg, np.float32)
    cc, ss = rope_tables()
    ident = np.eye(128, dtype=BF16NP)
    ones = np.ones((128, 128), dtype=BF16NP)
    xT = [np.ascontiguousarray(x[b].T).astype(BF16NP) for b in range(NB)]
    in_maps = []
    for b in range(NB):
        for tp in range(NTP):
            wkv = np.concatenate([
                Wk[tp * HD:(tp + 1) * HD, :].T,
                Wv[tp * HD:(tp + 1) * HD, :].T], axis=1)
            in_maps.append({
                "xT": xT[b],
                "wq": np.ascontiguousarray(
                    Wq[tp * DQ:(tp + 1) * DQ, :].T).astype(BF16NP),
                "wkv": np.ascontiguousarray(wkv).astype(BF16NP),
                "wo": np.ascontiguousarray(
                    Wo[:, tp * DQ:(tp + 1) * DQ].T).astype(BF16NP),
                "cc": cc,
                "ss": ss,
                "qgb": np.broadcast_to(
                    qg[tp * NQ:(tp + 1) * NQ][None, :], (128, NQ)).copy(),
                "ident": ident,
                "ones": ones,
            })
    return in_maps


def run(x, Wq, Wk, Wv, Wo, qg, trace=False, **trace_kwargs):
    nc = get_nc()
    in_maps = make_in_maps(x, Wq, Wk, Wv, Wo, qg)
    res = run_bass_kernel_spmd(nc, in_maps, core_ids=list(range(NCORES)),
                               trace=trace, **trace_kwargs)
    out = np.empty((NB, S, D), np.float32)
    for b in range(NB):
        acc = res.results[b * NTP]["outT"].astype(np.float32)
        for tp in range(1, NTP):
            acc = acc + res.results[b * NTP + tp]["outT"]
        out[b] = acc.T
    return out, res


def kernel(x, Wq, Wk, Wv, Wo, qg):
    out, _ = run(x, Wq, Wk, Wv, Wo, qg)
    return out
